# revision 6
# baseline (speedup 1.0000x reference)
"""FlowNetC correlation layer on 8 Trainium2 NeuronCores.

Math: out[b, d, y, x] = (1/256) * sum_c in1[b,c,y,x] * in2pad[b,c,y+dy,x+dx]
with (dy, dx) on a 21x21 stride-2 grid spanning [-20, 20], zero padding 20.

Strategy (per core = one batch sample; batch is exactly 8):
- Displacements have stride 2, so the problem splits into 4 independent parity
  classes. Each class: in1c [256, 32, 48] against a padded in2c [256, 52, 68]
  with stride-1 displacements dy', dx' in [0, 20].
- Gram band matmuls: for each class and group of 4 subsampled x-columns, run 4
  col-tiled matmuls (M=32 each, tile_position=(0, 32*xg)). Tile xg's stationary
  is in1c[:, :, x0] (32 ys columns); its moving tensor is the 21-wide window
  in2c[:, :, x0:x0+21] over the 32 VALID rows only (N = 32*21 = 672 in two
  PSUM chunks of 336) -- the 20 zero-padding rows are never streamed. PSUM
  partition p = 32*xg + ys holds rows [10,42) of the full 52-row band; the
  zero rows live as persistent zeros in the band buffers (memset once).
- Band buffers are two persistent tensors alternated per class so the zero
  regions survive reuse. One merged eviction per x-group writes the valid
  columns.
- De-shear: per-ys batched SBUF->SBUF DMAs (32 per class over 3 queues;
  partition-exact stride on xg, in-partition strides on g/d) produce dense
  [pixel, d] tiles with 441-element contiguous runs.
- The per-class pipeline is software-pipelined: class c's transposes and
  scatters are emitted after class c+1's gram, so the PE never stalls on the
  de-shear DMAs.
- TensorE transposes flip dense [pixel, d] tiles to [d, pixel], strided copies
  scatter into a d-major bf16 assembly buffer, and 4 output DMAs (gpsimd,
  bf16->f32 cast) write [441, 64, 96] with 24 KB contiguous runs per d.
- Matmul inputs are bf16 (1 cycle/column on the PE vs 4 for fp32); the 1/256
  normalization is folded into in1's bf16 cast exactly (exponent shift).
"""

import os
import sys

for _p in ("/opt/trn_rl_repo", "/root/.axon_site/_ro/trn_rl_repo"):
    if os.path.isdir(_p) and _p not in sys.path:
        sys.path.insert(0, _p)

from contextlib import ExitStack

import ml_dtypes
import numpy as np

import concourse.bacc as bacc
import concourse.bass as bass
import concourse.mybir as mybir
import concourse.tile as tile
from concourse.bass_utils import run_bass_kernel_spmd
from concourse.masks import make_identity

B, C, H, W = 8, 256, 64, 96
NYS, NXS = 32, 48          # subsampled class grid
RB, CB = 52, 68            # padded class grid (rows/cols)
RV = 32                    # valid rows per class (RB minus 2*10 zero rows)
ND = 441                   # displacements
WB = 1092                  # band width per xs-column (52 rows * 21 dx)
NG = 12                    # xs-column groups per class band
BP = NG * WB               # band per-partition size (13104)
DP = NG * ND               # dense per-partition size (5292)
NPIX = H * W               # 6144
DCHUNKS = [(0, 128), (128, 128), (256, 128), (384, 57)]
# valid-row psum chunks: rows [10,26) and [26,42), 336 cols each
GRAM_CHUNKS = [(10, 26), (26, 42)]

F32 = mybir.dt.float32
BF16 = mybir.dt.bfloat16


def build(reps=1, band_bf16=True, gram_only=False):
    """reps>1 repeats the whole pipeline in-NEFF (timing: cancels fixed overhead).
    gram_only: skip deshear/transpose/assembly (hardware experiment for PE time)."""
    BDT = BF16 if band_bf16 else F32
    nc = bacc.Bacc("TRN2", target_bir_lowering=False, debug=False, num_devices=8)
    in1p = nc.declare_dram_parameter("in1p", [4, 2, 128, NXS, NYS], BF16, isOutput=False)
    in2p = nc.declare_dram_parameter("in2p", [4, 2, 128, RV, CB], BF16, isOutput=False)
    outp = nc.declare_dram_parameter("out", [ND, H, W], F32, isOutput=True)

    with tile.TileContext(nc) as tc:
        with ExitStack() as ctx:
            const_pool = ctx.enter_context(tc.tile_pool(name="const", bufs=1))
            in2_pool = ctx.enter_context(tc.tile_pool(name="in2", bufs=2))
            dense_pool = ctx.enter_context(tc.tile_pool(name="dense", bufs=2))
            out_pool = ctx.enter_context(tc.tile_pool(name="outsb", bufs=1))
            pg_pool = ctx.enter_context(tc.tile_pool(name="pg", bufs=3, space="PSUM"))
            pt_pool = ctx.enter_context(tc.tile_pool(name="pt", bufs=2, space="PSUM"))

            ident = const_pool.tile([128, 128], BDT)
            make_identity(nc, ident)

            # resident in1: [c, k, cls, xs, ys], loaded per class so the first
            # gram group is not gated on the full 3 MB transfer
            in1_sb = const_pool.tile([128, 2, 4, NXS, NYS], BF16)
            CLS1 = NXS * NYS
            for cid in range(4):
                nc.scalar.dma_start(
                    out=bass.AP(in1_sb.tensor, in1_sb.offset + cid * CLS1,
                                [[2 * 4 * CLS1, 128], [4 * CLS1, 2], [1, CLS1]]),
                    in_=bass.AP(in1p, cid * 2 * 128 * CLS1,
                                [[CLS1, 128], [128 * CLS1, 2], [1, CLS1]]),
                )

            # persistent band buffers (double-buffered by class parity); the
            # zero-padding row regions are memset once and never rewritten
            bands = [const_pool.tile([128, NG, WB], BDT, tag=f"band{i}",
                                     name=f"band{i}") for i in range(2)]
            for bd in bands:
                for off, length in ((0, 210), (882, 210)):
                    nc.vector.memset(
                        bass.AP(bd.tensor, bd.offset + off,
                                [[BP, 128], [WB, NG], [1, length]]),
                        0,
                    )

            # persistent d-major assembly buffers, one per d-chunk
            out_sb = [out_pool.tile([128, NPIX], BF16, tag=f"out{dc}", name=f"out_sb{dc}")
                      for dc in range(4)]

            ev_flip = 0
            dma_flip = 0
            DSH_ENGS = (nc.sync, nc.scalar, nc.gpsimd)
            for rep in range(reps):
              state = None  # (cid, dense) awaiting transpose+scatter
              for cid in range(5):
                if cid < 4:
                    py, px = cid // 2, cid % 2
                    in2_sb = in2_pool.tile([128, 2, RV, CB], BF16)
                    nc.sync.dma_start(
                        out=bass.AP(in2_sb.tensor, in2_sb.offset,
                                    [[2 * RV * CB, 128], [RV * CB, 2], [1, RV * CB]]),
                        in_=bass.AP(in2p, cid * 2 * 128 * RV * CB,
                                    [[RV * CB, 128], [128 * RV * CB, 2], [1, RV * CB]]),
                    )
                    band = bands[(rep * 4 + cid) % 2]
                    for xsg in range(NG):
                        pg = pg_pool.tile([128, 2, 512], F32)
                        for k in range(2):
                            for xg in range(4):
                                x0 = 4 * xsg + xg
                                lhsT = in1_sb[:, k, cid, x0, :]
                                for ch, (r0, r1) in enumerate(GRAM_CHUNKS):
                                    ncols = (r1 - r0) * 21
                                    rhs = in2_sb[:, k, r0 - 10:r1 - 10, x0:x0 + 21]
                                    nc.tensor.matmul(
                                        pg[32 * xg:32 * (xg + 1), ch, 0:ncols],
                                        lhsT, rhs,
                                        start=(k == 0), stop=(k == 1),
                                        tile_position=(0, 32 * xg),
                                        skip_group_check=True,
                                    )
                        # one merged eviction of both psum chunks (672 cols)
                        src = pg[:, 0:2, 0:336]
                        dst = bass.AP(band.tensor,
                                      band.offset + xsg * WB + 210,
                                      [[BP, 128], [336, 2], [1, 336]])
                        if ev_flip % 2 == 0:
                            nc.vector.tensor_copy(out=dst, in_=src)
                        else:
                            nc.scalar.copy(out=dst, in_=src)
                        ev_flip += 1

                    if gram_only:
                        nc.vector.tensor_copy(out=out_sb[0][:, :512],
                                              in_=bass.AP(band.tensor, band.offset,
                                                          [[BP, 128], [1, 512]]))
                        continue

                    # de-shear: per-ys batched DMAs, band -> dense [pixel, d]
                    dense = dense_pool.tile([128, NG, ND], BDT)
                    for ys in range(NYS):
                        src = bass.AP(band.tensor, band.offset + ys * BP + 21 * ys,
                                      [[32 * BP, 4], [WB, NG], [1, ND]])
                        dst = bass.AP(dense.tensor, dense.offset + ys * DP,
                                      [[32 * DP, 4], [ND, NG], [1, ND]])
                        DSH_ENGS[dma_flip % 3].dma_start(out=dst, in_=src)
                        dma_flip += 1
                else:
                    dense = None

                # transpose + scatter for the PREVIOUS class (sw pipeline)
                if state is not None:
                    pcid, pdense = state
                    ppy, ppx = pcid // 2, pcid % 2
                    for dc, (d0, dcw) in enumerate(DCHUNKS):
                        for s in range(4):
                            pt = pt_pool.tile([128, 384], BDT)
                            for j in range(3):
                                nc.tensor.transpose(
                                    pt[0:dcw, j * 128:(j + 1) * 128],
                                    pdense[:, 3 * s + j, d0:d0 + dcw],
                                    ident[:],
                                )
                            ob = out_sb[dc]
                            src = bass.AP(pt.tensor, pt.offset,
                                          [[384, dcw], [128, 3], [32, 4], [1, 32]])
                            doff = 96 * ppy + ppx + 8 * (3 * s)
                            dst = bass.AP(ob.tensor, ob.offset + doff,
                                          [[NPIX, dcw], [8, 3], [2, 4], [192, 32]])
                            if ev_flip % 2 == 0:
                                nc.vector.tensor_copy(out=dst, in_=src)
                            else:
                                nc.scalar.copy(out=dst, in_=src)
                            ev_flip += 1
                state = (cid, dense) if cid < 4 else None

              # output: one cast DMA per d-chunk, 24KB contiguous runs per d
              for dc, (d0, dcw) in enumerate(DCHUNKS[:1] if gram_only else DCHUNKS):
                  ob = out_sb[dc]
                  nc.gpsimd.dma_start(
                      out=bass.AP(outp, d0 * NPIX, [[NPIX, dcw], [1, NPIX]]),
                      in_=bass.AP(ob.tensor, ob.offset, [[NPIX, dcw], [1, NPIX]]),
                  )

    nc.compile()
    return nc


def prep_inputs(input1, input2):
    """Host-side: parity split, column-pad, bf16 cast, fold 1/256 into in1."""
    in_maps = []
    for b in range(B):
        a1 = (input1[b].astype(np.float32) / 256.0).reshape(2, 128, H, W)
        a2 = input2[b].astype(np.float32).reshape(2, 128, H, W)
        in1p = np.empty((4, 2, 128, NXS, NYS), dtype=ml_dtypes.bfloat16)
        in2p = np.zeros((4, 2, 128, RV, CB), dtype=ml_dtypes.bfloat16)
        for cid in range(4):
            py, px = cid // 2, cid % 2
            in1p[cid] = a1[:, :, py::2, px::2].transpose(0, 1, 3, 2).astype(ml_dtypes.bfloat16)
            in2p[cid, :, :, :, 10:58] = a2[:, :, py::2, px::2].astype(ml_dtypes.bfloat16)
        in_maps.append({"in1p": in1p, "in2p": in2p})
    return in_maps


_NC = None


def get_nc():
    global _NC
    if _NC is None:
        _NC = build()
    return _NC


def kernel(input1, input2):
    nc = get_nc()
    in_maps = prep_inputs(np.asarray(input1), np.asarray(input2))
    r = run_bass_kernel_spmd(nc, in_maps, core_ids=list(range(8)))
    return np.stack([r.results[i]["out"] for i in range(B)]).astype(np.float32)
